# revision 1
# baseline (speedup 1.0000x reference)
"""Trainium2 Bass kernel for nn_LossSoftDice (soft-dice loss over 32 samples
of 1x512x512 probability/target maps).

Strategy: pure data parallel over the batch. Each of the 8 NeuronCores gets 4
samples (each sample = 262144 f32 elements, viewed as a [128, 2048] tile).
The device computes only per-partition statistics (everything else is
O(128) work done on host during the gather/unshard step):

  inter_p[p] = sum_f m1[p,f] * m2[p,f]   (DVE fused scalar_tensor_tensor)
  den_p[p]   = sum_f m1[p,f] + m2[p,f]   (one ACT pass over the [m2|m1] tile)
  maxp[p]    = max_f m2[p,f]             (DVE tensor_reduce)
  nsr_p[p]   = #{f : m1[p,f] > 0.5}      (2 samples: DVE tensor_scalar accum;
                                          2 samples: 2x-mode DVE compare +
                                          ACT accumulate, for engine balance)

Host combine (exact, matches the reference's acc branch):
  gmax = max_p maxp[p];  corr = N - nSR - K + 2A, where K (#elements equal to
  gmax) and A (#those with m1 > 0.5) come from scanning only the partitions
  whose maxp equals gmax (O(2048) per sample against the host-held inputs).
  score = 2*(inter+1)/(den+1);  score = 1 where corr == 1;
  loss = mean(1 - score)
"""

import os
import sys
import types

import numpy as np


def _ensure_concourse():
    try:
        import concourse.bass  # noqa: F401
    except ImportError:
        for p in ("/opt/trn_rl_repo", "/root/.axon_site/_ro/trn_rl_repo"):
            if os.path.isdir(p) and p not in sys.path:
                sys.path.insert(0, p)
        import concourse.bass  # noqa: F401


_ensure_concourse()

import concourse.bass as bass  # noqa: E402
import concourse.bacc as bacc  # noqa: E402
import concourse.tile as tile  # noqa: E402
from concourse import mybir  # noqa: E402
from concourse.bass_utils import run_bass_kernel_spmd  # noqa: E402
from concourse.vector_clock import ScopedClock  # noqa: E402

N_CORES = 8
B = 32                      # total batch
BPC = B // N_CORES          # samples per core
P = 128                     # partitions
F = 2048                    # free dim per partition (P*F = 512*512)

_MAX_WAITS_PER_INST = 1


def _patched_drain_and_barrier(self, tick_clock, wait_clock):
    """Walrus CoreV3Gen rejects CTRL instructions with >2 sem waits; the Tile
    tail drain can carry many. Split them one-per-NoOp before the drain."""
    nc = self.nc
    drain_inst = nc.sync.drain()
    wait_clock.add_sem_waits(
        drain_inst.ins, ScopedClock({None: tick_clock.global_clock})
    )
    si = drain_inst.ins.sync_info
    if si is not None and si.on_wait and len(si.on_wait) > _MAX_WAITS_PER_INST:
        waits = list(si.on_wait)
        si.on_wait = waits[:_MAX_WAITS_PER_INST]
        insts = nc.cur_bb.bb.instructions
        assert insts[-1] is drain_inst.ins
        nops = []
        for w in waits[_MAX_WAITS_PER_INST:]:
            nop_inst = nc.sync.nop(nofuse=True, hint="drain_wait_split")
            if nop_inst.ins.sync_info is None:
                nop_inst.ins.sync_info = mybir.SyncInfo(on_wait=[], on_update=[])
            nop_inst.ins.sync_info.on_wait.append(w)
            nops.append(insts.pop())
        d = insts.pop()
        insts.extend(nops)
        insts.append(d)

    nc.all_engine_barrier()
    assert self.sems is not None
    popped = nc._tile_sem_poison_stack.pop()
    assert popped is self._sem_poison
    nc.clear_and_free_semaphores(list(self.sems.allocated().values()))
    nc.all_engine_barrier()


def _slim_drain_and_barrier(self, tick_clock, wait_clock):
    # Same as TileContext._drain_and_barrier but without the second
    # all-engine barrier: NRT itself waits for every engine to halt before
    # the NEFF can be re-executed, so the sem clear does not need another
    # intra-NEFF barrier after it. (Bacc.compile legalizes multi-waits.)
    nc = self.nc
    drain_inst = nc.sync.drain()
    wait_clock.add_sem_waits(
        drain_inst.ins, ScopedClock({None: tick_clock.global_clock})
    )
    nc.all_engine_barrier()
    assert self.sems is not None
    popped = nc._tile_sem_poison_stack.pop()
    assert popped is self._sem_poison
    nc.clear_and_free_semaphores(list(self.sems.allocated().values()))


tile.TileContext._drain_and_barrier = _slim_drain_and_barrier


def _install_ntff_hook_module():
    """bass_utils imports antenv.axon_hooks when trace=True under axon; this
    container's antenv lacks that module. Recreate it from the boot helper."""
    if "antenv.axon_hooks" in sys.modules:
        return
    try:
        import trn_agent_boot.trn_boot as tb

        hook = tb._ntff_profile_via_ctypes("/opt/axon/libaxon_pjrt.so")
    except Exception:
        hook = None
    m = types.ModuleType("antenv.axon_hooks")
    m.get_axon_ntff_profile_hook = lambda: hook
    m.set_axon_ntff_profile_hook = lambda h: None
    sys.modules["antenv.axon_hooks"] = m


_STAT_NAMES = ("inter", "den", "maxp", "nsr")


def _build_nc():
    nc = bacc.Bacc("TRN2", debug=False)
    f32 = mybir.dt.float32
    probs = nc.dram_tensor("probs", [BPC, P, F], f32, kind="ExternalInput").ap()
    targets = nc.dram_tensor("targets", [BPC, P, F], f32, kind="ExternalInput").ap()
    stats_out = nc.dram_tensor(
        "stats", [P, 4 * BPC], f32, kind="ExternalOutput"
    ).ap()

    A = mybir.AluOpType
    with tile.TileContext(nc) as tc:
        with (
            tc.tile_pool(name="m1", bufs=BPC) as m1_pool,
            tc.tile_pool(name="m2", bufs=BPC) as m2_pool,
            tc.tile_pool(name="scr", bufs=1) as scr_pool,
            tc.tile_pool(name="sr", bufs=3) as sr_pool,
            tc.tile_pool(name="stats", bufs=1) as stats_pool,
        ):
            mds = []
            for s in range(BPC):
                md = m1_pool.tile([P, 2 * F], f32, tag="md", name=f"md{s}")
                # m2 in the low half (sync ring), m1 in the high half
                # (scalar ring) - two HWDGE rings dispatch in parallel.
                nc.sync.dma_start(md[:, 0:F], targets[s])
                nc.scalar.dma_start(md[:, F : 2 * F], probs[s])
                mds.append(md)

            dve_scr = scr_pool.tile([P, F], f32, tag="dve_scr")
            act_scr = scr_pool.tile([P, 2 * F], f32, tag="act_scr")
            st_tile = stats_pool.tile(
                [P, 4 * BPC], f32, tag="st", name="st_all"
            )
            st = {
                name: st_tile[:, j * BPC : (j + 1) * BPC]
                for j, name in enumerate(_STAT_NAMES)
            }

            for s in range(BPC):
                md = mds[s]
                m2 = md[:, 0:F]
                m1 = md[:, F : 2 * F]
                c = slice(s, s + 1)
                # per-partition max of targets (needs only m2 -> starts first)
                nc.vector.tensor_reduce(
                    st["maxp"][:, c], m2, mybir.AxisListType.X, A.max
                )
                # denominator: per-partition sum of (m2|m1) in one ACT pass
                nc.scalar.activation(
                    act_scr[:], md[:], mybir.ActivationFunctionType.Copy,
                    accum_out=st["den"][:, c],
                )
                if s >= BPC - 2:
                    # balance: last sample counts SR on DVE (accum variant)
                    sr = sr_pool.tile([P, F], f32, tag="sr")
                    nc.vector.tensor_scalar(
                        sr[:], m1, 0.5, None, A.is_gt, A.add,
                        accum_out=st["nsr"][:, c],
                    )
                else:
                    # SR = m1 > 0.5 (plain tensor_scalar -> 2x DVE mode),
                    # counted on the scalar engine
                    sr = sr_pool.tile([P, F], f32, tag="sr")
                    nc.vector.tensor_scalar(sr[:], m1, 0.5, None, A.is_gt)
                    nc.scalar.activation(
                        act_scr[:, 0:F], sr[:], mybir.ActivationFunctionType.Copy,
                        accum_out=st["nsr"][:, c],
                    )
                # intersection per partition (+ throwaway product tile)
                nc.vector.scalar_tensor_tensor(
                    out=dve_scr[:],
                    in0=m1,
                    scalar=1.0,
                    in1=m2,
                    op0=A.mult,
                    op1=A.mult,
                    accum_out=st["inter"][:, c],
                )

            nc.sync.dma_start(stats_out, st_tile[:])

    nc.compile()
    return nc


def _shard_inputs(probs, targets):
    probs = np.ascontiguousarray(np.asarray(probs, dtype=np.float32)).reshape(B, P, F)
    targets = np.ascontiguousarray(np.asarray(targets, dtype=np.float32)).reshape(
        B, P, F
    )
    in_maps = []
    for i in range(N_CORES):
        sl = slice(i * BPC, (i + 1) * BPC)
        in_maps.append(
            {
                "probs": np.ascontiguousarray(probs[sl]),
                "targets": np.ascontiguousarray(targets[sl]),
            }
        )
    return in_maps


def _combine(results, probs, targets):
    """Exact host-side combine of per-partition stats -> scalar loss.

    corr_b = N - nSR - K + 2A with K (#elements == global max) and
    A (#those with m1 > 0.5) recovered by scanning only the partitions
    that attain the global max (O(2048) per sample, exact)."""
    inter = np.empty(B)
    den = np.empty(B)
    corr = np.empty(B)
    N = float(P * F)
    for i in range(N_CORES):
        r = results[i]["stats"]
        col = {name: r[:, j * BPC : (j + 1) * BPC] for j, name in enumerate(_STAT_NAMES)}
        for s in range(BPC):
            b = i * BPC + s
            inter[b] = col["inter"][:, s].astype(np.float64).sum()
            den[b] = col["den"][:, s].astype(np.float64).sum()
            nsr = col["nsr"][:, s].astype(np.float64).sum()
            maxp = col["maxp"][:, s]
            gmax = maxp.max()
            K = A = 0
            for p in np.nonzero(maxp == gmax)[0]:
                hit = targets[b, p, :] == gmax
                K += int(hit.sum())
                A += int((hit & (probs[b, p, :] > 0.5)).sum())
            corr[b] = N - nsr - K + 2 * A
    score = 2.0 * (inter + 1.0) / (den + 1.0)
    score = np.where(corr == 1.0, 1.0, score)
    return np.array(np.mean(1.0 - score), dtype=np.float32)


def _run(probs, targets, trace=False, tmpdir=None):
    _install_ntff_hook_module()
    nc = _build_nc()
    in_maps = _shard_inputs(probs, targets)
    res = run_bass_kernel_spmd(
        nc, in_maps, list(range(N_CORES)), trace=trace, tmpdir=tmpdir
    )
    pr = np.asarray(probs, dtype=np.float32).reshape(B, P, F)
    tg = np.asarray(targets, dtype=np.float32).reshape(B, P, F)
    out = _combine(res.results, pr, tg)
    return out, res


def kernel(probs, targets):
    out, _ = _run(probs, targets)
    return out



# revision 4
# speedup vs baseline: 1.0000x; 1.0000x over previous
"""Trainium2 Bass kernel for nn_LossSoftDice (soft-dice loss over 32 samples
of 1x512x512 probability/target maps).

Strategy: pure data parallel over the batch. Each of the 8 NeuronCores gets 4
samples (each sample = 262144 f32 elements, viewed as a [128, 2048] tile).
The device computes only per-partition statistics (everything else is
O(128) work done on host during the gather/unshard step).

Engine balance (per core, ~19us DMA stream at ~435 GB/s is the floor):
  DVE   : maxp[p]  = max_f m2[p,f]           (tensor_reduce, per chunk)
          inter[p] = sum_f m1[p,f]*m2[p,f]   (scalar_tensor_tensor accum)
          + one final [4,512] PSUM->stats reduce for the PE partial sums
  ACT   : sgn[p]   = sum_f sign(m1[p,f]-0.5) (Sign activation w/ accum;
                                              nsr = (N + sgn)/2, exact mod
                                              0.5-ties fixed up on host)
          sm2[p]   = sum_f m2[p,f]           (Copy activation w/ accum)
  PE    : sm1[s]   = sum_pf m1[p,f]          (ones-column stationary matmul,
                                              accumulated in PSUM [4,512])
  DMA   : m2 chunks on the sync HWDGE queue, m1 chunks on the gpsimd SWDGE
          queue; two queues round-robin at packet granularity -> ~435 GB/s.

Host combine (exact, matches the reference's acc branch):
  den = sm1[s] + sum_p sm2;  score = 2*(inter+1)/(den+1)
  corr_b = N - nSR - K + 2A with K (#elements == global max) and A (#those
  with m1 > 0.5) recovered by scanning only the partitions that attain the
  global max (O(2048) per sample, exact); score = 1 where corr == 1;
  loss = mean(1 - score)
"""

import os
import sys
import types

import numpy as np


def _ensure_concourse():
    try:
        import concourse.bass  # noqa: F401
    except ImportError:
        for p in ("/opt/trn_rl_repo", "/root/.axon_site/_ro/trn_rl_repo"):
            if os.path.isdir(p) and p not in sys.path:
                sys.path.insert(0, p)
        import concourse.bass  # noqa: F401


_ensure_concourse()

import concourse.bass as bass  # noqa: E402
import concourse.bacc as bacc  # noqa: E402
import concourse.tile as tile  # noqa: E402
from concourse import mybir  # noqa: E402
from concourse.bass_utils import run_bass_kernel_spmd  # noqa: E402
from concourse.vector_clock import ScopedClock  # noqa: E402

N_CORES = 8
B = 32                      # total batch
BPC = B // N_CORES          # samples per core
P = 128                     # partitions
F = 2048                    # free dim per partition (P*F = 512*512)

# chunk layout per sample: samples 0-2 in 2 chunks of 1024 cols, sample 3 in
# 4 chunks of 512 cols (smaller tail chunks shrink the post-stream backlog)
CHUNKS = []
for _s in range(BPC - 1):
    CHUNKS.append((_s, 0, 1024))
    CHUNKS.append((_s, 1024, 1024))
for _c in range(4):
    CHUNKS.append((BPC - 1, _c * 512, 512))
NCH = len(CHUNKS)

# use the gpsimd SWDGE queue for the m1 stream (second DMA queue that does
# not occupy the ACT engine with dispatch instructions)
M1_ON_GPSIMD = True


def _slim_drain_and_barrier(self, tick_clock, wait_clock):
    # Same as TileContext._drain_and_barrier but without the second
    # all-engine barrier: NRT itself waits for every engine to halt before
    # the NEFF can be re-executed, so the sem clear does not need another
    # intra-NEFF barrier after it. (Bacc.compile legalizes multi-waits.)
    nc = self.nc
    drain_inst = nc.sync.drain()
    wait_clock.add_sem_waits(
        drain_inst.ins, ScopedClock({None: tick_clock.global_clock})
    )
    nc.all_engine_barrier()
    assert self.sems is not None
    popped = nc._tile_sem_poison_stack.pop()
    assert popped is self._sem_poison
    nc.clear_and_free_semaphores(list(self.sems.allocated().values()))


tile.TileContext._drain_and_barrier = _slim_drain_and_barrier


def _install_ntff_hook_module():
    """bass_utils imports antenv.axon_hooks when trace=True under axon; this
    container's antenv lacks that module. Recreate it from the boot helper."""
    if "antenv.axon_hooks" in sys.modules:
        return
    try:
        import trn_agent_boot.trn_boot as tb

        hook = tb._ntff_profile_via_ctypes("/opt/axon/libaxon_pjrt.so")
    except Exception:
        hook = None
    m = types.ModuleType("antenv.axon_hooks")
    m.get_axon_ntff_profile_hook = lambda: hook
    m.set_axon_ntff_profile_hook = lambda h: None
    sys.modules["antenv.axon_hooks"] = m


# stats tile layout (f32 columns): per-chunk slots for maxp/inter/sgn/sm2,
# then one column holding the PE-reduced per-sample m1 sums in rows 0..3
N_SLOT = NCH
COL_MAXP = 0
COL_INTER = N_SLOT
COL_SGN = 2 * N_SLOT
COL_SM2 = 3 * N_SLOT
COL_SM1 = 4 * N_SLOT
N_COLS = 4 * N_SLOT + 1


def _build_nc():
    nc = bacc.Bacc("TRN2", debug=False)
    f32 = mybir.dt.float32
    probs = nc.dram_tensor("probs", [BPC, P, F], f32, kind="ExternalInput").ap()
    targets = nc.dram_tensor("targets", [BPC, P, F], f32, kind="ExternalInput").ap()
    stats_out = nc.dram_tensor("stats", [P, N_COLS], f32, kind="ExternalOutput").ap()

    A = mybir.AluOpType
    AF = mybir.ActivationFunctionType
    with tile.TileContext(nc) as tc:
        with (
            tc.tile_pool(name="inp", bufs=1) as inp_pool,
            tc.tile_pool(name="scr", bufs=1) as scr_pool,
            tc.tile_pool(name="stats", bufs=1) as stats_pool,
            tc.psum_pool(name="psum", bufs=1) as psum_pool,
        ):
            # per-chunk input tiles, all resident so DMA never stalls
            m2t, m1t = [], []
            for i, (s, c0, w) in enumerate(CHUNKS):
                m2c = inp_pool.tile([P, w], f32, tag=f"m2_{i}", name=f"m2_{i}")
                m1c = inp_pool.tile([P, w], f32, tag=f"m1_{i}", name=f"m1_{i}")
                m2t.append(m2c)
                m1t.append(m1c)

            # stationary weights: one [128,16] tile; cols 4s+s hold 1.0 so
            # lhsT slice [:, 4s:4s+4] routes sample s's column sums to PSUM
            # partition row s (other rows accumulate zeros)
            ones = scr_pool.tile([P, 4 * BPC], f32, tag="ones")
            nc.gpsimd.memset(ones[:], 0.0)
            for s in range(BPC):
                nc.gpsimd.memset(ones[:, 4 * s + s : 4 * s + s + 1], 1.0)
            neg_half = scr_pool.tile([P, 1], f32, tag="neg_half")
            nc.gpsimd.memset(neg_half[:], -0.5)

            dve_scr = scr_pool.tile([P, 1024], f32, tag="dve_scr")
            sgn_scr = scr_pool.tile([P, 1024], f32, tag="sgn_scr")
            cp_scr = scr_pool.tile([P, 1024], f32, tag="cp_scr")
            st = stats_pool.tile([P, N_COLS], f32, tag="st", name="st_all")
            psum_t = psum_pool.tile([BPC, 512], f32, tag="acc")

            # input DMAs: m2 chunks on sync HWDGE, m1 chunks on gpsimd SWDGE
            for i, (s, c0, w) in enumerate(CHUNKS):
                nc.sync.dma_start(m2t[i][:], targets[s][:, c0 : c0 + w])
                if M1_ON_GPSIMD:
                    nc.gpsimd.dma_start(m1t[i][:], probs[s][:, c0 : c0 + w])
                else:
                    nc.scalar.dma_start(m1t[i][:], probs[s][:, c0 : c0 + w])

            # compute, in chunk-arrival order
            n_mm = 0
            for i, (s, c0, w) in enumerate(CHUNKS):
                m2 = m2t[i][:]
                m1 = m1t[i][:]
                c = slice(i, i + 1)
                # DVE: per-partition max of targets chunk
                nc.vector.tensor_reduce(
                    st[:, COL_MAXP + i : COL_MAXP + i + 1],
                    m2,
                    mybir.AxisListType.X,
                    A.max,
                )
                # ACT: count(m1 > 0.5) via sign(m1 - 0.5) accumulation
                nc.scalar.activation(
                    sgn_scr[:, 0:w], m1, AF.Sign, bias=neg_half[:],
                    accum_out=st[:, COL_SGN + i : COL_SGN + i + 1],
                )
                # ACT: sum of m2 chunk
                nc.scalar.activation(
                    cp_scr[:, 0:w], m2, AF.Copy,
                    accum_out=st[:, COL_SM2 + i : COL_SM2 + i + 1],
                )
                # PE: sum of m1 chunk -> PSUM row s (512-col matmuls)
                for q0 in range(0, w, 512):
                    nc.tensor.matmul(
                        psum_t[:, :],
                        ones[:, 4 * s : 4 * s + 4],
                        m1[:, q0 : q0 + 512],
                        start=(n_mm == 0),
                        stop=(n_mm == 15),
                        skip_group_check=True,
                    )
                    n_mm += 1
                # DVE: intersection accumulate
                nc.vector.scalar_tensor_tensor(
                    out=dve_scr[:, 0:w],
                    in0=m1,
                    scalar=1.0,
                    in1=m2,
                    op0=A.mult,
                    op1=A.mult,
                    accum_out=st[:, COL_INTER + i : COL_INTER + i + 1],
                )

            # DVE: fold PE partial sums -> per-sample m1 totals in rows 0..3
            nc.vector.tensor_reduce(
                st[0:BPC, COL_SM1 : COL_SM1 + 1],
                psum_t[:, :],
                mybir.AxisListType.X,
                A.add,
            )

            nc.sync.dma_start(stats_out, st[:])

    nc.compile()
    return nc


def _shard_inputs(probs, targets):
    probs = np.ascontiguousarray(np.asarray(probs, dtype=np.float32)).reshape(B, P, F)
    targets = np.ascontiguousarray(np.asarray(targets, dtype=np.float32)).reshape(
        B, P, F
    )
    in_maps = []
    for i in range(N_CORES):
        sl = slice(i * BPC, (i + 1) * BPC)
        in_maps.append(
            {
                "probs": np.ascontiguousarray(probs[sl]),
                "targets": np.ascontiguousarray(targets[sl]),
            }
        )
    return in_maps


def _combine(results, probs, targets):
    """Exact host-side combine of per-partition stats -> scalar loss."""
    inter = np.empty(B)
    den = np.empty(B)
    corr = np.empty(B)
    N = float(P * F)
    for i in range(N_CORES):
        r = results[i]["stats"].astype(np.float64)
        maxp_sl = r[:, COL_MAXP : COL_MAXP + N_SLOT]
        inter_sl = r[:, COL_INTER : COL_INTER + N_SLOT]
        sgn_sl = r[:, COL_SGN : COL_SGN + N_SLOT]
        sm2_sl = r[:, COL_SM2 : COL_SM2 + N_SLOT]
        sm1 = r[:, COL_SM1]
        for s in range(BPC):
            b = i * BPC + s
            idx = [j for j, (cs, _, _) in enumerate(CHUNKS) if cs == s]
            inter[b] = inter_sl[:, idx].sum()
            den[b] = sm1[s] + sm2_sl[:, idx].sum()
            # nsr from the sign sums: count = (n + S)/2 per partition-chunk;
            # an exact 0.5 contributes sign()=0 making (n + S) odd -> rescan
            # that partition-chunk against host data (exact, O(w))
            nsr = 0.0
            for j in idx:
                cs, c0, w = CHUNKS[j]
                tot = w + sgn_sl[:, j]
                odd = np.nonzero(np.round(tot).astype(np.int64) & 1)[0]
                for p in odd:
                    tot[p] = 2 * np.count_nonzero(
                        probs[b, p, c0 : c0 + w] > 0.5
                    )
                nsr += tot.sum() / 2.0
            # per-partition max of targets over this sample's chunk slots
            maxp = maxp_sl[:, idx].max(axis=1)
            gmax = maxp.max()
            K = Acnt = 0
            for p in np.nonzero(maxp == gmax)[0]:
                hit = targets[b, p, :] == np.float32(gmax)
                K += int(hit.sum())
                Acnt += int((hit & (probs[b, p, :] > 0.5)).sum())
            corr[b] = N - nsr - K + 2 * Acnt
    score = 2.0 * (inter + 1.0) / (den + 1.0)
    score = np.where(corr == 1.0, 1.0, score)
    return np.array(np.mean(1.0 - score), dtype=np.float32)


def _run(probs, targets, trace=False, tmpdir=None):
    _install_ntff_hook_module()
    nc = _build_nc()
    in_maps = _shard_inputs(probs, targets)
    res = run_bass_kernel_spmd(
        nc, in_maps, list(range(N_CORES)), trace=trace, tmpdir=tmpdir
    )
    pr = np.asarray(probs, dtype=np.float32).reshape(B, P, F)
    tg = np.asarray(targets, dtype=np.float32).reshape(B, P, F)
    out = _combine(res.results, pr, tg)
    return out, res


def kernel(probs, targets):
    out, _ = _run(probs, targets)
    return out
